# revision 36
# baseline (speedup 1.0000x reference)
# DeepseekV3MoECalibrate Trainium2 kernel (8 NeuronCores, expert-parallel).
#
# Sharding: 32 experts -> 4 per core; shared expert split along the 2I=2048
# intermediate dim (256 per core, processed as one 2-i-tile pseudo-expert);
# tokens replicated; partial outputs summed with an on-device AllReduce.
#
# Numerics: router (sigmoid top-k selection) in fp32; expert/shared MLP
# matmuls in fp32r (full PE rate at N=512, ~1.6e-4 rel err); combine weights
# applied before the down-projection so each entry's contribution is a plain
# sum accumulated in SBUF.
#
# Schedule: weight transposition for entry k+1 is software-pipelined into
# entry k's down-projection (stage-3) matmul stream so the PE never crawls
# through an evacuation-gated transpose phase.
from contextlib import ExitStack

import numpy as np

import concourse.bass as bass
import concourse.tile as tile
from concourse import bacc, mybir
from concourse.masks import make_identity

F32 = mybir.dt.float32
F32R = mybir.dt.float32r
AF = mybir.ActivationFunctionType
OP = mybir.AluOpType
AX = mybir.AxisListType

N_CORES = 8
T, H, I, E = 1024, 1024, 512, 32
E_LOC = E // N_CORES          # 4 experts per core
ISH = 2 * I // N_CORES        # 256 shared-intermediate rows per core
TT = T // 128                 # 8 token tiles
HK = H // 128                 # 8 h k-tiles
IK = I // 128                 # 4 i-tiles per expert
SK = ISH // 128               # 2 shared i-tiles
NH = H // 512                 # 2 h halves (stage-3 rhs)
TH = T // 512                 # 2 t halves (stage-1 rhs)


def build_module(use_collective=True, num_devices=N_CORES):
    nc = bacc.Bacc("TRN2", target_bir_lowering=False, debug=False,
                   num_devices=num_devices)

    x_d = nc.dram_tensor("x", [T, H], F32, kind="ExternalInput")
    gw_d = nc.dram_tensor("gw", [E, H], F32, kind="ExternalInput")
    gb_d = nc.dram_tensor("gb", [1, E], F32, kind="ExternalInput")
    wsel_d = nc.dram_tensor("wsel", [E, E_LOC], F32, kind="ExternalInput")
    eg_d = nc.dram_tensor("eg", [E_LOC, I, H], F32, kind="ExternalInput")
    eu_d = nc.dram_tensor("eu", [E_LOC, I, H], F32, kind="ExternalInput")
    ed_d = nc.dram_tensor("ed", [E_LOC, H, I], F32, kind="ExternalInput")
    sg_d = nc.dram_tensor("sg", [ISH, H], F32, kind="ExternalInput")
    su_d = nc.dram_tensor("su", [ISH, H], F32, kind="ExternalInput")
    sd_d = nc.dram_tensor("sd", [H, ISH], F32, kind="ExternalInput")
    out_rows = T // num_devices if use_collective else T
    out_d = nc.dram_tensor("out", [out_rows, H], F32, kind="ExternalOutput")

    with tile.TileContext(nc) as tc, ExitStack() as ctx:
        const = ctx.enter_context(tc.tile_pool(name="const", bufs=1))
        sbr = ctx.enter_context(tc.tile_pool(name="router", bufs=2))
        xpool = ctx.enter_context(tc.tile_pool(name="xt", bufs=1))
        wg_pool = ctx.enter_context(tc.tile_pool(name="wgt", bufs=8))
        wu_pool = ctx.enter_context(tc.tile_pool(name="wut", bufs=8))
        wd_pool = ctx.enter_context(tc.tile_pool(name="wdt", bufs=4))
        nat_pool = ctx.enter_context(tc.tile_pool(name="nat", bufs=3))
        ats_pool = ctx.enter_context(tc.tile_pool(name="ats", bufs=4))
        tmp_pool = ctx.enter_context(tc.tile_pool(name="tmp", bufs=2))
        acc_pool = ctx.enter_context(tc.tile_pool(name="acc", bufs=1))
        dram = ctx.enter_context(tc.tile_pool(name="dram", bufs=1, space="DRAM"))

        ps_a = ctx.enter_context(tc.tile_pool(name="ps_a", bufs=2, space="PSUM"))
        ps_s = ctx.enter_context(tc.tile_pool(name="ps_s", bufs=4, space="PSUM"))
        ps_o = ctx.enter_context(tc.tile_pool(name="ps_o", bufs=2, space="PSUM"))

        ident_f = const.tile([128, 128], F32, name="ident_f")
        make_identity(nc, ident_f[:])
        ident_r = const.tile([128, 128], F32R, name="ident_r")
        nc.vector.tensor_copy(ident_r[:], ident_f[:])

        # ------------- router prep ------------------------------------------
        gw_sb = sbr.tile([E, H], F32, name="gw_sb")
        nc.sync.dma_start(gw_sb[:], gw_d[:])
        gb_sb = sbr.tile([1, E], F32, name="gb_sb")
        nc.sync.dma_start(gb_sb[:], gb_d[:])
        wsel_sb = sbr.tile([E, E_LOC], F32R, name="wsel_sb")
        nc.sync.dma_start(wsel_sb[:], wsel_d[:].bitcast(F32R))

        gwT = [sbr.tile([128, E], F32, name=f"gwT{h}") for h in range(HK)]
        for ht in range(HK):
            p = ps_a.tile([128, 512], F32, name=f"gwp{ht}", tag="ps_a")
            nc.tensor.transpose(p[:, 0:E], gw_sb[:, ht * 128:(ht + 1) * 128],
                                ident_f[0:E, 0:E])
            nc.vector.tensor_copy(gwT[ht][:], p[:, 0:E])

        ones_f = const.tile([1, 128], F32, name="ones_f")
        nc.vector.memset(ones_f[:], 1.0)
        bias_bc = sbr.tile([128, E], F32, name="bias_bc")
        pb = ps_a.tile([128, 512], F32, name="pb", tag="ps_a")
        nc.tensor.matmul(pb[:, 0:E], ones_f[:], gb_sb[:], start=True, stop=True)
        nc.vector.tensor_copy(bias_bc[:], pb[:, 0:E])

        # ------------- persistent activations / accumulators -----------------
        xT_r = xpool.tile([128, HK * T], F32R, name="xTr")
        xview = xT_r[:].rearrange("p (h t) -> p h t", h=HK)
        wT_r = sbr.tile([E, T], F32R, name="wT_r")
        lgs = [sbr.tile([128, E], F32, name=f"lgs{tt}") for tt in range(TT)]
        out_acc = [acc_pool.tile([128, H], F32, name=f"oacc{tt}")
                   for tt in range(TT)]

        # ------------- weight prep (load + transpose), batched ---------------
        def make_prep(ei, kind, idx, ike):
            """Allocate destination tiles; return (wgTd, wdT, batches) where
            each batch closure emits a couple of PE transposes + one copy."""
            wgTd = {}
            batches = []
            for gu in ("g", "u"):
                pool = wg_pool if gu == "g" else wu_pool
                tagw = "wgt" if gu == "g" else "wut"
                dst = [pool.tile([128, ike * 128], F32R,
                                 name=f"w{gu}T{ei}_{h}", tag=tagw)
                       for h in range(HK)]
                wgTd[gu] = dst
                ihalf = 2 if kind == "expert" else 1
                per = ike // ihalf
                for ih in range(ihalf):
                    state = {}

                    def load_half(gu=gu, ih=ih, per=per, state=state):
                        natm = [nat_pool.tile([128, H], F32R,
                                              name=f"nat{ei}{gu}{ih}_{i}",
                                              tag="natgu", bufs=3)
                                for i in range(per)]
                        state["natm"] = natm
                        for i in range(per):
                            it = ih * per + i
                            if kind == "expert":
                                mat = eg_d if gu == "g" else eu_d
                                src = mat[idx, it * 128:(it + 1) * 128, :]
                            else:
                                mat = sg_d if gu == "g" else su_d
                                src = mat[it * 128:(it + 1) * 128, :]
                            nc.sync.dma_start(natm[i][:], src.bitcast(F32R))

                    for ht in range(HK):
                        def b(gu=gu, ih=ih, ht=ht, per=per, state=state,
                              dst=dst, lh=load_half):
                            if ht == 0:
                                lh()
                            natm = state["natm"]
                            p = ps_a.tile([128, 512], F32,
                                          name=f"wp{ei}{gu}{ih}{ht}", tag="ps_a")
                            for i in range(per):
                                nc.tensor.transpose(
                                    p[:, i * 128:(i + 1) * 128].bitcast(F32R),
                                    natm[i][:, ht * 128:(ht + 1) * 128],
                                    ident_r[:])
                            nc.scalar.copy(
                                dst[ht][:, ih * per * 128:(ih + 1) * per * 128],
                                p[:, 0:per * 128].bitcast(F32R))
                        batches.append(b)

            wdT = [wd_pool.tile([128, H], F32R, name=f"wdT{ei}_{i}", tag="wdt")
                   for i in range(ike)]
            for htq in range(HK // 4):
                state = {}

                def load_dn(htq=htq, state=state):
                    dn_nat = [nat_pool.tile([128, ike * 128], F32R,
                                            name=f"dnat{ei}_{htq}_{j}",
                                            tag="natdn", bufs=6)
                              for j in range(4)]
                    state["dn"] = dn_nat
                    for j in range(4):
                        ht = htq * 4 + j
                        if kind == "expert":
                            src = ed_d[idx, ht * 128:(ht + 1) * 128, :]
                        else:
                            src = sd_d[ht * 128:(ht + 1) * 128, :]
                        nc.sync.dma_start(dn_nat[j][:], src.bitcast(F32R))

                for it in range(ike):
                    def b(htq=htq, it=it, state=state, wdT=wdT, ld=load_dn):
                        if it == 0:
                            ld()
                        dn_nat = state["dn"]
                        p = ps_a.tile([128, 512], F32,
                                      name=f"dp{ei}_{htq}_{it}", tag="ps_a")
                        for j in range(4):
                            nc.tensor.transpose(
                                p[:, j * 128:(j + 1) * 128].bitcast(F32R),
                                dn_nat[j][:, it * 128:(it + 1) * 128],
                                ident_r[:])
                        nc.vector.tensor_copy(
                            wdT[it][:, htq * 512:(htq + 1) * 512].bitcast(F32R),
                            p[:].bitcast(F32R))
                    batches.append(b)
            return wgTd, wdT, batches

        # ------------- stage 1 / stage 3 -------------------------------------
        def stage1(ei, kind, ike, wgTd, wb):
            aTs = [ats_pool.tile([128, T], F32R, name=f"aTs{ei}_{i}", tag="ats")
                   for i in range(ike)]
            for th in range(TH):
                for it in range(ike):
                    gp = ps_s.tile([128, 512], F32, name=f"gp{ei}_{it}_{th}",
                                   tag="ps_s")
                    up = ps_s.tile([128, 512], F32, name=f"up{ei}_{it}_{th}",
                                   tag="ps_s")
                    for ht in range(HK):
                        nc.tensor.matmul(
                            gp[:], wgTd["g"][ht][:, it * 128:(it + 1) * 128],
                            xT_r[:, ht * T + th * 512: ht * T + (th + 1) * 512],
                            start=(ht == 0), stop=(ht == HK - 1))
                    for ht in range(HK):
                        nc.tensor.matmul(
                            up[:], wgTd["u"][ht][:, it * 128:(it + 1) * 128],
                            xT_r[:, ht * T + th * 512: ht * T + (th + 1) * 512],
                            start=(ht == 0), stop=(ht == HK - 1))
                    sg_t = tmp_pool.tile([128, 512], F32, name=f"sl{ei}{it}{th}",
                                         tag="silu", bufs=3)
                    nc.scalar.activation(sg_t[:], gp[:], AF.Silu)
                    dst = aTs[it][:, th * 512:(th + 1) * 512].bitcast(F32R)
                    if kind == "expert":
                        nc.vector.tensor_tensor(sg_t[:], sg_t[:], up[:], OP.mult)
                        nc.vector.tensor_tensor(
                            dst, sg_t[:], wb[:, th * 512:(th + 1) * 512],
                            OP.mult)
                    else:
                        nc.vector.tensor_tensor(dst, sg_t[:], up[:], OP.mult)
            return aTs

        def stage3_groups(ei, ike, aTs, wdT):
            groups = []
            for tt in range(TT):
                for hh in range(NH):
                    def g(tt=tt, hh=hh):
                        op = ps_o.tile([128, 512], F32, name=f"op{ei}_{tt}_{hh}",
                                       tag="ps_o")
                        for it in range(ike):
                            nc.tensor.matmul(
                                op[:], aTs[it][:, tt * 128:(tt + 1) * 128],
                                wdT[it][:, hh * 512:(hh + 1) * 512],
                                start=(it == 0), stop=(it == ike - 1))
                        dst = out_acc[tt][:, hh * 512:(hh + 1) * 512]
                        if ei == 0:
                            nc.vector.tensor_copy(dst, op[:])
                        else:
                            nc.vector.tensor_tensor(dst, dst, op[:], OP.add)
                    groups.append(g)
            return groups

        def emit_interleaved(groups, batches):
            bi = 0
            n = len(groups)
            for gi, g in enumerate(groups):
                g()
                # front-load: finish all batches by ~2/3 through the groups
                want = min(len(batches), (gi + 1) * len(batches) * 2 // n)
                while bi < want:
                    batches[bi]()
                    bi += 1
            while bi < len(batches):
                batches[bi]()
                bi += 1

        # ------------- routing ------------------------------------------------
        def do_routing(tt):
            S = sbr.tile([128, E], F32, name=f"S{tt}", tag="S")
            nc.scalar.activation(S[:], lgs[tt][:], AF.Sigmoid)
            SC = sbr.tile([128, E], F32, name=f"SC{tt}", tag="SC")
            nc.vector.tensor_tensor(SC[:], S[:], bias_bc[:], OP.add)
            topg = sbr.tile([128, E], F32, name=f"topg{tt}", tag="topg")
            for g in range(4):
                nc.vector.max(topg[:, 8 * g:8 * g + 8], SC[:, 8 * g:8 * g + 8])
            gs8 = sbr.tile([128, 8], F32, name=f"gs8{tt}", tag="gs8")
            nc.vector.memset(gs8[:], -1e30)
            tg = topg[:].rearrange("p (g k) -> p g k", k=8)
            nc.vector.tensor_tensor(gs8[:, 0:4], tg[:, :, 0], tg[:, :, 1], OP.add)
            gtop = sbr.tile([128, 8], F32, name=f"gtop{tt}", tag="gtop")
            nc.vector.max(gtop[:], gs8[:])
            gmask = sbr.tile([128, 4], F32, name=f"gmask{tt}", tag="gmask")
            nc.vector.tensor_scalar(gmask[:], gs8[:, 0:4], gtop[:, 1:2], None,
                                    OP.is_ge)
            SCm = sbr.tile([128, E], F32, name=f"SCm{tt}", tag="SCm")
            nc.vector.tensor_tensor(
                SCm[:].rearrange("p (g k) -> p g k", k=8),
                SC[:].rearrange("p (g k) -> p g k", k=8),
                gmask[:].rearrange("p (g k) -> p g k", k=1).broadcast_to(
                    [128, 4, 8]),
                OP.mult)
            etop = sbr.tile([128, 8], F32, name=f"etop{tt}", tag="etop")
            nc.vector.max(etop[:], SCm[:])
            sel = sbr.tile([128, E], F32, name=f"sel{tt}", tag="sel")
            nc.vector.tensor_scalar(sel[:], SCm[:], etop[:, 7:8], None, OP.is_ge)
            wr = sbr.tile([128, E], F32, name=f"wr{tt}", tag="wr")
            nc.vector.tensor_tensor(wr[:], S[:], sel[:], OP.mult)
            den = sbr.tile([128, 1], F32, name=f"den{tt}", tag="den")
            nc.vector.reduce_sum(den[:], wr[:], axis=AX.X)
            nc.vector.tensor_scalar(den[:], den[:], 1.0 / 2.5, None, OP.mult)
            dinv = sbr.tile([128, 1], F32, name=f"dinv{tt}", tag="dinv")
            nc.vector.reciprocal(dinv[:], den[:])
            wt = sbr.tile([128, E], F32, name=f"wt{tt}", tag="wt")
            nc.vector.tensor_scalar(wt[:], wr[:], dinv[:], None, OP.mult)
            tp = ps_a.tile([128, 512], F32, name=f"tw{tt}", tag="ps_a")
            nc.tensor.transpose(tp[0:E, 0:128], wt[:], ident_f[:])
            nc.vector.tensor_copy(wT_r[:, tt * 128:(tt + 1) * 128].bitcast(F32R),
                                  tp[0:E, 0:128].bitcast(F32R))

        wb_tiles = {}
        wsel_bcs = {}

        def wb_th(e, th):
            if e not in wsel_bcs:
                wselbc = tmp_pool.tile([E, 128], F32R, name=f"wsb{e}",
                                       tag="wselbc")
                nc.vector.tensor_copy(
                    wselbc[:], wsel_sb[:, e:e + 1].broadcast_to([E, 128]))
                wsel_bcs[e] = wselbc
            if e not in wb_tiles:
                wb_tiles[e] = tmp_pool.tile([128, T], F32, name=f"wbx{e}",
                                            tag="wb")
            wb = wb_tiles[e]
            p = ps_a.tile([128, 512], F32, name=f"wbp{e}_{th}", tag="ps_a")
            nc.tensor.matmul(p[:], wsel_bcs[e][:],
                             wT_r[:, th * 512:(th + 1) * 512],
                             start=True, stop=True)
            nc.vector.tensor_copy(wb[:, th * 512:(th + 1) * 512], p[:])

        def make_wb(e):
            wb_th(e, 0)
            wb_th(e, 1)

        # ================= emission schedule =================================
        # Phase X: x transposes + router logits, with expert-0's weight prep
        # interleaved.  Routing is split by token half so expert-0's stage-1
        # th=0 can start as soon as tokens 0..511 are routed.  The shared
        # entry runs LAST so its (pool-serialized) weight prep overlaps the
        # final expert instead of the congested startup window.
        e0_prep = make_prep(0, "expert", 0, IK)
        bi = 0
        for tt in range(TT):
            xn = nat_pool.tile([128, H], F32, name=f"xn{tt}", tag="xn", bufs=2)
            (nc.sync if tt % 2 == 0 else nc.gpsimd).dma_start(
                xn[:], x_d[tt * 128:(tt + 1) * 128, :])
            xfb = tmp_pool.tile([128, H], F32, name=f"xfb{tt}", tag="xfb")
            for hq in range(HK // 4):
                p = ps_s.tile([128, 512], F32, name=f"xp{tt}_{hq}", tag="ps_s")
                for j in range(4):
                    ht = hq * 4 + j
                    nc.tensor.transpose(
                        p[:, j * 128:(j + 1) * 128],
                        xn[:, ht * 128:(ht + 1) * 128], ident_f[:])
                nc.scalar.copy(xfb[:, hq * 512:(hq + 1) * 512], p[:])
                nc.gpsimd.tensor_copy(
                    xview[:, hq * 4:(hq + 1) * 4,
                          tt * 128:(tt + 1) * 128].bitcast(F32R),
                    xfb[:, hq * 512:(hq + 1) * 512]
                    .rearrange("p (h t) -> p h t", h=4).bitcast(F32R))

            lg = ps_a.tile([128, 512], F32, name=f"lg{tt}", tag="ps_a")
            for ht in range(HK):
                nc.tensor.matmul(lg[:, 0:E], xfb[:, ht * 128:(ht + 1) * 128],
                                 gwT[ht][:],
                                 start=(ht == 0), stop=(ht == HK - 1))
            nc.scalar.copy(lgs[tt][:], lg[:, 0:E])

            want = (tt + 1) * len(e0_prep[2]) // TT
            while bi < want:
                e0_prep[2][bi]()
                bi += 1

        for tt in range(TT // 2):
            do_routing(tt)
        wb_th(0, 0)
        wb_th(1, 0)
        for tt in range(TT // 2, TT):
            do_routing(tt)
        wb_th(0, 1)
        wb_th(1, 1)

        # Entry pipeline: stage-3 of entry k interleaves entry k+1's prep.
        order = [("expert", e, IK) for e in range(E_LOC - 1)] + \
                [("shared", 0, SK), ("expert", E_LOC - 1, IK)]
        prev = e0_prep
        for k, (kind, idx, ike) in enumerate(order):
            wgTd, wdT, _ = prev
            wb = wb_tiles.get(idx) if kind == "expert" else None
            aTs = stage1(k, kind, ike, wgTd, wb)
            if kind == "expert" and idx + 2 < E_LOC:
                make_wb(idx + 2)
            if k + 1 < len(order):
                knd, nidx, nike = order[k + 1]
                nxt = make_prep(k + 1, knd, nidx, nike)
            else:
                nxt = None
            emit_interleaved(stage3_groups(k, ike, aTs, wdT),
                             nxt[2] if nxt else [])
            prev = nxt

        # ------------- ReduceScatter + output -------------------------------
        # Each core keeps its 128-token shard of the summed output; the host
        # concatenates the 8 shards.  RS moves ~30% less wire traffic than an
        # AllReduce of the full [T, H].
        if use_collective:
            bin_t = dram.tile([T, H], F32, name="rsin")
            bout_t = dram.tile([out_rows, H], F32, name="rsout")
            for tt in range(TT):
                nc.sync.dma_start(bin_t[tt * 128:(tt + 1) * 128, :],
                                  out_acc[tt][:])
            nc.gpsimd.collective_compute(
                "ReduceScatter", OP.add,
                replica_groups=[list(range(num_devices))],
                ins=[bin_t.opt()], outs=[bout_t.opt()])
            nc.sync.dma_start(out_d[:], bout_t[:])
        else:
            for tt in range(TT):
                nc.sync.dma_start(out_d[tt * 128:(tt + 1) * 128, :],
                                  out_acc[tt][:])
    nc.compile()
    return nc


_NC_CACHE = {}


def _get_module():
    key = "spmd"
    if key not in _NC_CACHE:
        _NC_CACHE[key] = build_module(use_collective=True, num_devices=N_CORES)
    return _NC_CACHE[key]


def make_in_maps(hidden_states, gate_w, gate_bias, expert_gate, expert_up,
                 expert_down, shared_gate, shared_up, shared_down):
    x = np.ascontiguousarray(
        np.asarray(hidden_states, np.float32).reshape(T, H))
    gw = np.ascontiguousarray(np.asarray(gate_w, np.float32))
    gb = np.ascontiguousarray(np.asarray(gate_bias, np.float32).reshape(1, E))
    in_maps = []
    for c in range(N_CORES):
        lo, hi = c * E_LOC, (c + 1) * E_LOC
        sel = np.zeros((E, E_LOC), np.float32)
        for j in range(E_LOC):
            sel[lo + j, j] = 1.0
        in_maps.append({
            "x": x, "gw": gw, "gb": gb, "wsel": sel,
            "eg": np.ascontiguousarray(np.asarray(expert_gate, np.float32)[lo:hi]),
            "eu": np.ascontiguousarray(np.asarray(expert_up, np.float32)[lo:hi]),
            "ed": np.ascontiguousarray(np.asarray(expert_down, np.float32)[lo:hi]),
            "sg": np.ascontiguousarray(
                np.asarray(shared_gate, np.float32)[c * ISH:(c + 1) * ISH]),
            "su": np.ascontiguousarray(
                np.asarray(shared_up, np.float32)[c * ISH:(c + 1) * ISH]),
            "sd": np.ascontiguousarray(
                np.asarray(shared_down, np.float32)[:, c * ISH:(c + 1) * ISH]),
        })
    return in_maps


def kernel(hidden_states, gate_w, gate_bias, expert_gate, expert_up,
           expert_down, shared_gate, shared_up, shared_down):
    import os
    # The axon NTFF trace hook is absent in this container; make sure the
    # PJRT execute path never tries to use it.
    os.environ.setdefault("BASS_NEVER_TRACE", "1")
    from concourse.bass_utils import run_bass_kernel_spmd
    nc = _get_module()
    in_maps = make_in_maps(hidden_states, gate_w, gate_bias, expert_gate,
                           expert_up, expert_down, shared_gate, shared_up,
                           shared_down)
    res = run_bass_kernel_spmd(nc, in_maps, core_ids=list(range(N_CORES)))
    out = np.concatenate([np.asarray(res.results[c]["out"], np.float32)
                          for c in range(N_CORES)], axis=0)
    return out.reshape(np.asarray(hidden_states).shape)


# revision 37
# speedup vs baseline: 1.1565x; 1.1565x over previous
# DeepseekV3MoECalibrate Trainium2 kernel (8 NeuronCores, expert-parallel).
#
# Sharding: 32 experts -> 4 per core; shared expert split along the 2I=2048
# intermediate dim (256 per core, processed as one 2-i-tile pseudo-expert);
# tokens replicated; partial outputs summed with an on-device AllReduce.
#
# Numerics: router (sigmoid top-k selection) in fp32; expert/shared MLP
# matmuls in fp32r (full PE rate at N=512, ~1.6e-4 rel err); combine weights
# applied before the down-projection so each entry's contribution is a plain
# sum accumulated in SBUF.
#
# Schedule: weight transposition for entry k+1 is software-pipelined into
# entry k's down-projection (stage-3) matmul stream so the PE never crawls
# through an evacuation-gated transpose phase.
from contextlib import ExitStack

import numpy as np

import concourse.bass as bass
import concourse.tile as tile
from concourse import bacc, mybir
from concourse.masks import make_identity

F32 = mybir.dt.float32
F32R = mybir.dt.float32r
AF = mybir.ActivationFunctionType
OP = mybir.AluOpType
AX = mybir.AxisListType

N_CORES = 8
T, H, I, E = 1024, 1024, 512, 32
E_LOC = E // N_CORES          # 4 experts per core
ISH = 2 * I // N_CORES        # 256 shared-intermediate rows per core
TT = T // 128                 # 8 token tiles
HK = H // 128                 # 8 h k-tiles
IK = I // 128                 # 4 i-tiles per expert
SK = ISH // 128               # 2 shared i-tiles
NH = H // 512                 # 2 h halves (stage-3 rhs)
TH = T // 512                 # 2 t halves (stage-1 rhs)


def build_module(use_collective=True, num_devices=N_CORES):
    nc = bacc.Bacc("TRN2", target_bir_lowering=False, debug=False,
                   num_devices=num_devices)

    x_d = nc.dram_tensor("x", [T, H], F32, kind="ExternalInput")
    gw_d = nc.dram_tensor("gw", [E, H], F32, kind="ExternalInput")
    gb_d = nc.dram_tensor("gb", [1, E], F32, kind="ExternalInput")
    wsel_d = nc.dram_tensor("wsel", [E, E_LOC], F32, kind="ExternalInput")
    eg_d = nc.dram_tensor("eg", [E_LOC, I, H], F32, kind="ExternalInput")
    eu_d = nc.dram_tensor("eu", [E_LOC, I, H], F32, kind="ExternalInput")
    ed_d = nc.dram_tensor("ed", [E_LOC, H, I], F32, kind="ExternalInput")
    sg_d = nc.dram_tensor("sg", [ISH, H], F32, kind="ExternalInput")
    su_d = nc.dram_tensor("su", [ISH, H], F32, kind="ExternalInput")
    sd_d = nc.dram_tensor("sd", [H, ISH], F32, kind="ExternalInput")
    out_rows = T // num_devices if use_collective else T
    out_d = nc.dram_tensor("out", [out_rows, H], F32, kind="ExternalOutput")

    with tile.TileContext(nc) as tc, ExitStack() as ctx:
        const = ctx.enter_context(tc.tile_pool(name="const", bufs=1))
        sbr = ctx.enter_context(tc.tile_pool(name="router", bufs=2))
        xpool = ctx.enter_context(tc.tile_pool(name="xt", bufs=1))
        wg_pool = ctx.enter_context(tc.tile_pool(name="wgt", bufs=8))
        wu_pool = ctx.enter_context(tc.tile_pool(name="wut", bufs=8))
        wd_pool = ctx.enter_context(tc.tile_pool(name="wdt", bufs=4))
        nat_pool = ctx.enter_context(tc.tile_pool(name="nat", bufs=3))
        ats_pool = ctx.enter_context(tc.tile_pool(name="ats", bufs=4))
        tmp_pool = ctx.enter_context(tc.tile_pool(name="tmp", bufs=2))
        acc_pool = ctx.enter_context(tc.tile_pool(name="acc", bufs=1))
        dram = ctx.enter_context(tc.tile_pool(name="dram", bufs=1, space="DRAM"))

        ps_a = ctx.enter_context(tc.tile_pool(name="ps_a", bufs=2, space="PSUM"))
        ps_s = ctx.enter_context(tc.tile_pool(name="ps_s", bufs=4, space="PSUM"))
        ps_o = ctx.enter_context(tc.tile_pool(name="ps_o", bufs=2, space="PSUM"))

        ident_f = const.tile([128, 128], F32, name="ident_f")
        make_identity(nc, ident_f[:])
        ident_r = const.tile([128, 128], F32R, name="ident_r")
        nc.vector.tensor_copy(ident_r[:], ident_f[:])

        # ------------- router prep ------------------------------------------
        gw_sb = sbr.tile([E, H], F32, name="gw_sb")
        nc.sync.dma_start(gw_sb[:], gw_d[:])
        gb_sb = sbr.tile([1, E], F32, name="gb_sb")
        nc.sync.dma_start(gb_sb[:], gb_d[:])
        wsel_sb = sbr.tile([E, E_LOC], F32R, name="wsel_sb")
        nc.sync.dma_start(wsel_sb[:], wsel_d[:].bitcast(F32R))

        gwT = [sbr.tile([128, E], F32, name=f"gwT{h}") for h in range(HK)]
        for ht in range(HK):
            p = ps_a.tile([128, 512], F32, name=f"gwp{ht}", tag="ps_a")
            nc.tensor.transpose(p[:, 0:E], gw_sb[:, ht * 128:(ht + 1) * 128],
                                ident_f[0:E, 0:E])
            nc.vector.tensor_copy(gwT[ht][:], p[:, 0:E])

        ones_f = const.tile([1, 128], F32, name="ones_f")
        nc.vector.memset(ones_f[:], 1.0)
        bias_bc = sbr.tile([128, E], F32, name="bias_bc")
        pb = ps_a.tile([128, 512], F32, name="pb", tag="ps_a")
        nc.tensor.matmul(pb[:, 0:E], ones_f[:], gb_sb[:], start=True, stop=True)
        nc.vector.tensor_copy(bias_bc[:], pb[:, 0:E])

        # ------------- persistent activations / accumulators -----------------
        xT_r = xpool.tile([128, HK * T], F32R, name="xTr")
        xview = xT_r[:].rearrange("p (h t) -> p h t", h=HK)
        wT_r = sbr.tile([E, T], F32R, name="wT_r")
        lgs = [sbr.tile([128, E], F32, name=f"lgs{tt}") for tt in range(TT)]
        out_acc = [acc_pool.tile([128, H], F32, name=f"oacc{tt}")
                   for tt in range(TT)]

        # ------------- weight prep (load + transpose), batched ---------------
        def make_prep(ei, kind, idx, ike):
            """Allocate destination tiles; return (wgTd, wdT, batches) where
            each batch closure emits a couple of PE transposes + one copy."""
            wgTd = {}
            batches = []
            for gu in ("g", "u"):
                pool = wg_pool if gu == "g" else wu_pool
                tagw = "wgt" if gu == "g" else "wut"
                dst = [pool.tile([128, ike * 128], F32R,
                                 name=f"w{gu}T{ei}_{h}", tag=tagw)
                       for h in range(HK)]
                wgTd[gu] = dst
                ihalf = 2 if kind == "expert" else 1
                per = ike // ihalf
                for ih in range(ihalf):
                    state = {}

                    def load_half(gu=gu, ih=ih, per=per, state=state):
                        natm = [nat_pool.tile([128, H], F32R,
                                              name=f"nat{ei}{gu}{ih}_{i}",
                                              tag="natgu", bufs=3)
                                for i in range(per)]
                        state["natm"] = natm
                        for i in range(per):
                            it = ih * per + i
                            if kind == "expert":
                                mat = eg_d if gu == "g" else eu_d
                                src = mat[idx, it * 128:(it + 1) * 128, :]
                            else:
                                mat = sg_d if gu == "g" else su_d
                                src = mat[it * 128:(it + 1) * 128, :]
                            nc.sync.dma_start(natm[i][:], src.bitcast(F32R))

                    for ht in range(HK):
                        def b(gu=gu, ih=ih, ht=ht, per=per, state=state,
                              dst=dst, lh=load_half):
                            if ht == 0:
                                lh()
                            natm = state["natm"]
                            p = ps_a.tile([128, 512], F32,
                                          name=f"wp{ei}{gu}{ih}{ht}", tag="ps_a")
                            for i in range(per):
                                nc.tensor.transpose(
                                    p[:, i * 128:(i + 1) * 128].bitcast(F32R),
                                    natm[i][:, ht * 128:(ht + 1) * 128],
                                    ident_r[:])
                            nc.scalar.copy(
                                dst[ht][:, ih * per * 128:(ih + 1) * per * 128],
                                p[:, 0:per * 128].bitcast(F32R))
                        batches.append(b)

            wdT = [wd_pool.tile([128, H], F32R, name=f"wdT{ei}_{i}", tag="wdt")
                   for i in range(ike)]
            for htq in range(HK // 4):
                state = {}

                def load_dn(htq=htq, state=state):
                    dn_nat = [nat_pool.tile([128, ike * 128], F32R,
                                            name=f"dnat{ei}_{htq}_{j}",
                                            tag="natdn", bufs=6)
                              for j in range(4)]
                    state["dn"] = dn_nat
                    for j in range(4):
                        ht = htq * 4 + j
                        if kind == "expert":
                            src = ed_d[idx, ht * 128:(ht + 1) * 128, :]
                        else:
                            src = sd_d[ht * 128:(ht + 1) * 128, :]
                        nc.sync.dma_start(dn_nat[j][:], src.bitcast(F32R))

                for it in range(ike):
                    def b(htq=htq, it=it, state=state, wdT=wdT, ld=load_dn):
                        if it == 0:
                            ld()
                        dn_nat = state["dn"]
                        p = ps_a.tile([128, 512], F32,
                                      name=f"dp{ei}_{htq}_{it}", tag="ps_a")
                        for j in range(4):
                            nc.tensor.transpose(
                                p[:, j * 128:(j + 1) * 128].bitcast(F32R),
                                dn_nat[j][:, it * 128:(it + 1) * 128],
                                ident_r[:])
                        nc.vector.tensor_copy(
                            wdT[it][:, htq * 512:(htq + 1) * 512].bitcast(F32R),
                            p[:].bitcast(F32R))
                    batches.append(b)
            return wgTd, wdT, batches

        # ------------- stage 1 / stage 3 -------------------------------------
        def stage1(ei, kind, ike, wgTd, wb):
            aTs = [ats_pool.tile([128, T], F32R, name=f"aTs{ei}_{i}", tag="ats")
                   for i in range(ike)]
            for th in range(TH):
                for it in range(ike):
                    gp = ps_s.tile([128, 512], F32, name=f"gp{ei}_{it}_{th}",
                                   tag="ps_s")
                    up = ps_s.tile([128, 512], F32, name=f"up{ei}_{it}_{th}",
                                   tag="ps_s")
                    for ht in range(HK):
                        nc.tensor.matmul(
                            gp[:], wgTd["g"][ht][:, it * 128:(it + 1) * 128],
                            xT_r[:, ht * T + th * 512: ht * T + (th + 1) * 512],
                            start=(ht == 0), stop=(ht == HK - 1))
                    for ht in range(HK):
                        nc.tensor.matmul(
                            up[:], wgTd["u"][ht][:, it * 128:(it + 1) * 128],
                            xT_r[:, ht * T + th * 512: ht * T + (th + 1) * 512],
                            start=(ht == 0), stop=(ht == HK - 1))
                    sg_t = tmp_pool.tile([128, 512], F32, name=f"sl{ei}{it}{th}",
                                         tag="silu", bufs=3)
                    nc.scalar.activation(sg_t[:], gp[:], AF.Silu)
                    dst = aTs[it][:, th * 512:(th + 1) * 512].bitcast(F32R)
                    if kind == "expert":
                        nc.vector.tensor_tensor(sg_t[:], sg_t[:], up[:], OP.mult)
                        nc.vector.tensor_tensor(
                            dst, sg_t[:], wb[:, th * 512:(th + 1) * 512],
                            OP.mult)
                    else:
                        nc.vector.tensor_tensor(dst, sg_t[:], up[:], OP.mult)
            return aTs

        def stage3_groups(ei, ike, aTs, wdT):
            groups = []
            for tt in range(TT):
                for hh in range(NH):
                    def g(tt=tt, hh=hh):
                        op = ps_o.tile([128, 512], F32, name=f"op{ei}_{tt}_{hh}",
                                       tag="ps_o")
                        for it in range(ike):
                            nc.tensor.matmul(
                                op[:], aTs[it][:, tt * 128:(tt + 1) * 128],
                                wdT[it][:, hh * 512:(hh + 1) * 512],
                                start=(it == 0), stop=(it == ike - 1))
                        dst = out_acc[tt][:, hh * 512:(hh + 1) * 512]
                        if ei == 0:
                            nc.vector.tensor_copy(dst, op[:])
                        else:
                            nc.vector.tensor_tensor(dst, dst, op[:], OP.add)
                    groups.append(g)
            return groups

        def emit_interleaved(groups, batches, front=2):
            bi = 0
            n = len(groups)
            for gi, g in enumerate(groups):
                g()
                want = min(len(batches), (gi + 1) * len(batches) * front // n)
                while bi < want:
                    batches[bi]()
                    bi += 1
            while bi < len(batches):
                batches[bi]()
                bi += 1

        # ------------- routing ------------------------------------------------
        def do_routing(tt):
            S = sbr.tile([128, E], F32, name=f"S{tt}", tag="S")
            nc.scalar.activation(S[:], lgs[tt][:], AF.Sigmoid)
            SC = sbr.tile([128, E], F32, name=f"SC{tt}", tag="SC")
            nc.vector.tensor_tensor(SC[:], S[:], bias_bc[:], OP.add)
            topg = sbr.tile([128, E], F32, name=f"topg{tt}", tag="topg")
            for g in range(4):
                nc.vector.max(topg[:, 8 * g:8 * g + 8], SC[:, 8 * g:8 * g + 8])
            gs8 = sbr.tile([128, 8], F32, name=f"gs8{tt}", tag="gs8")
            nc.vector.memset(gs8[:], -1e30)
            tg = topg[:].rearrange("p (g k) -> p g k", k=8)
            nc.vector.tensor_tensor(gs8[:, 0:4], tg[:, :, 0], tg[:, :, 1], OP.add)
            gtop = sbr.tile([128, 8], F32, name=f"gtop{tt}", tag="gtop")
            nc.vector.max(gtop[:], gs8[:])
            gmask = sbr.tile([128, 4], F32, name=f"gmask{tt}", tag="gmask")
            nc.vector.tensor_scalar(gmask[:], gs8[:, 0:4], gtop[:, 1:2], None,
                                    OP.is_ge)
            SCm = sbr.tile([128, E], F32, name=f"SCm{tt}", tag="SCm")
            nc.vector.tensor_tensor(
                SCm[:].rearrange("p (g k) -> p g k", k=8),
                SC[:].rearrange("p (g k) -> p g k", k=8),
                gmask[:].rearrange("p (g k) -> p g k", k=1).broadcast_to(
                    [128, 4, 8]),
                OP.mult)
            etop = sbr.tile([128, 8], F32, name=f"etop{tt}", tag="etop")
            nc.vector.max(etop[:], SCm[:])
            sel = sbr.tile([128, E], F32, name=f"sel{tt}", tag="sel")
            nc.vector.tensor_scalar(sel[:], SCm[:], etop[:, 7:8], None, OP.is_ge)
            wr = sbr.tile([128, E], F32, name=f"wr{tt}", tag="wr")
            nc.vector.tensor_tensor(wr[:], S[:], sel[:], OP.mult)
            den = sbr.tile([128, 1], F32, name=f"den{tt}", tag="den")
            nc.vector.reduce_sum(den[:], wr[:], axis=AX.X)
            nc.vector.tensor_scalar(den[:], den[:], 1.0 / 2.5, None, OP.mult)
            dinv = sbr.tile([128, 1], F32, name=f"dinv{tt}", tag="dinv")
            nc.vector.reciprocal(dinv[:], den[:])
            wt = sbr.tile([128, E], F32, name=f"wt{tt}", tag="wt")
            nc.vector.tensor_scalar(wt[:], wr[:], dinv[:], None, OP.mult)
            tp = ps_a.tile([128, 512], F32, name=f"tw{tt}", tag="ps_a")
            nc.tensor.transpose(tp[0:E, 0:128], wt[:], ident_f[:])
            nc.vector.tensor_copy(wT_r[:, tt * 128:(tt + 1) * 128].bitcast(F32R),
                                  tp[0:E, 0:128].bitcast(F32R))

        wb_tiles = {}
        wsel_bcs = {}

        def wb_th(e, th):
            if e not in wsel_bcs:
                wselbc = tmp_pool.tile([E, 128], F32R, name=f"wsb{e}",
                                       tag="wselbc")
                nc.vector.tensor_copy(
                    wselbc[:], wsel_sb[:, e:e + 1].broadcast_to([E, 128]))
                wsel_bcs[e] = wselbc
            if e not in wb_tiles:
                wb_tiles[e] = tmp_pool.tile([128, T], F32, name=f"wbx{e}",
                                            tag="wb")
            wb = wb_tiles[e]
            p = ps_a.tile([128, 512], F32, name=f"wbp{e}_{th}", tag="ps_a")
            nc.tensor.matmul(p[:], wsel_bcs[e][:],
                             wT_r[:, th * 512:(th + 1) * 512],
                             start=True, stop=True)
            nc.vector.tensor_copy(wb[:, th * 512:(th + 1) * 512], p[:])

        def make_wb(e):
            wb_th(e, 0)
            wb_th(e, 1)

        # ================= emission schedule =================================
        # Phase X: x transposes + router logits, with expert-0's weight prep
        # interleaved.  Routing is split by token half so expert-0's stage-1
        # th=0 can start as soon as tokens 0..511 are routed.  The shared
        # entry runs LAST so its (pool-serialized) weight prep overlaps the
        # final expert instead of the congested startup window.
        e0_prep = make_prep(0, "expert", 0, IK)
        bi = 0
        for tt in range(TT):
            xn = nat_pool.tile([128, H], F32, name=f"xn{tt}", tag="xn", bufs=2)
            (nc.sync if tt % 2 == 0 else nc.gpsimd).dma_start(
                xn[:], x_d[tt * 128:(tt + 1) * 128, :])
            xfb = tmp_pool.tile([128, H], F32, name=f"xfb{tt}", tag="xfb")
            for hq in range(HK // 4):
                p = ps_s.tile([128, 512], F32, name=f"xp{tt}_{hq}", tag="ps_s")
                for j in range(4):
                    ht = hq * 4 + j
                    nc.tensor.transpose(
                        p[:, j * 128:(j + 1) * 128],
                        xn[:, ht * 128:(ht + 1) * 128], ident_f[:])
                nc.scalar.copy(xfb[:, hq * 512:(hq + 1) * 512], p[:])
                nc.gpsimd.tensor_copy(
                    xview[:, hq * 4:(hq + 1) * 4,
                          tt * 128:(tt + 1) * 128].bitcast(F32R),
                    xfb[:, hq * 512:(hq + 1) * 512]
                    .rearrange("p (h t) -> p h t", h=4).bitcast(F32R))

            lg = ps_a.tile([128, 512], F32, name=f"lg{tt}", tag="ps_a")
            for ht in range(HK):
                nc.tensor.matmul(lg[:, 0:E], xfb[:, ht * 128:(ht + 1) * 128],
                                 gwT[ht][:],
                                 start=(ht == 0), stop=(ht == HK - 1))
            nc.scalar.copy(lgs[tt][:], lg[:, 0:E])

            want = (tt + 1) * len(e0_prep[2]) // TT
            while bi < want:
                e0_prep[2][bi]()
                bi += 1

        for tt in range(TT // 2):
            do_routing(tt)
        wb_th(0, 0)
        wb_th(1, 0)
        for tt in range(TT // 2, TT):
            do_routing(tt)
        wb_th(0, 1)
        wb_th(1, 1)

        # Entry pipeline: stage-3 of entry k interleaves entry k+1's prep.
        order = [("expert", e, IK) for e in range(E_LOC - 1)] + \
                [("shared", 0, SK), ("expert", E_LOC - 1, IK)]
        prev = e0_prep
        for k, (kind, idx, ike) in enumerate(order):
            wgTd, wdT, _ = prev
            wb = wb_tiles.get(idx) if kind == "expert" else None
            aTs = stage1(k, kind, ike, wgTd, wb)
            if kind == "expert" and idx + 2 < E_LOC:
                make_wb(idx + 2)
            if k + 1 < len(order):
                knd, nidx, nike = order[k + 1]
                nxt = make_prep(k + 1, knd, nidx, nike)
            else:
                nxt = None
            emit_interleaved(stage3_groups(k, ike, aTs, wdT),
                             nxt[2] if nxt else [],
                             front=2 if k < len(order) - 2 else 1)
            prev = nxt

        # ------------- ReduceScatter + output -------------------------------
        # Each core keeps its 128-token shard of the summed output; the host
        # concatenates the 8 shards.  RS moves ~30% less wire traffic than an
        # AllReduce of the full [T, H].
        if use_collective:
            bin_t = dram.tile([T, H], F32, name="rsin")
            bout_t = dram.tile([out_rows, H], F32, name="rsout")
            for tt in range(TT):
                nc.sync.dma_start(bin_t[tt * 128:(tt + 1) * 128, :],
                                  out_acc[tt][:])
            nc.gpsimd.collective_compute(
                "ReduceScatter", OP.add,
                replica_groups=[list(range(num_devices))],
                ins=[bin_t.opt()], outs=[bout_t.opt()])
            nc.sync.dma_start(out_d[:], bout_t[:])
        else:
            for tt in range(TT):
                nc.sync.dma_start(out_d[tt * 128:(tt + 1) * 128, :],
                                  out_acc[tt][:])
    nc.compile()
    return nc


_NC_CACHE = {}


def _get_module():
    key = "spmd"
    if key not in _NC_CACHE:
        _NC_CACHE[key] = build_module(use_collective=True, num_devices=N_CORES)
    return _NC_CACHE[key]


def make_in_maps(hidden_states, gate_w, gate_bias, expert_gate, expert_up,
                 expert_down, shared_gate, shared_up, shared_down):
    x = np.ascontiguousarray(
        np.asarray(hidden_states, np.float32).reshape(T, H))
    gw = np.ascontiguousarray(np.asarray(gate_w, np.float32))
    gb = np.ascontiguousarray(np.asarray(gate_bias, np.float32).reshape(1, E))
    in_maps = []
    for c in range(N_CORES):
        lo, hi = c * E_LOC, (c + 1) * E_LOC
        sel = np.zeros((E, E_LOC), np.float32)
        for j in range(E_LOC):
            sel[lo + j, j] = 1.0
        in_maps.append({
            "x": x, "gw": gw, "gb": gb, "wsel": sel,
            "eg": np.ascontiguousarray(np.asarray(expert_gate, np.float32)[lo:hi]),
            "eu": np.ascontiguousarray(np.asarray(expert_up, np.float32)[lo:hi]),
            "ed": np.ascontiguousarray(np.asarray(expert_down, np.float32)[lo:hi]),
            "sg": np.ascontiguousarray(
                np.asarray(shared_gate, np.float32)[c * ISH:(c + 1) * ISH]),
            "su": np.ascontiguousarray(
                np.asarray(shared_up, np.float32)[c * ISH:(c + 1) * ISH]),
            "sd": np.ascontiguousarray(
                np.asarray(shared_down, np.float32)[:, c * ISH:(c + 1) * ISH]),
        })
    return in_maps


def kernel(hidden_states, gate_w, gate_bias, expert_gate, expert_up,
           expert_down, shared_gate, shared_up, shared_down):
    import os
    # The axon NTFF trace hook is absent in this container; make sure the
    # PJRT execute path never tries to use it.
    os.environ.setdefault("BASS_NEVER_TRACE", "1")
    from concourse.bass_utils import run_bass_kernel_spmd
    nc = _get_module()
    in_maps = make_in_maps(hidden_states, gate_w, gate_bias, expert_gate,
                           expert_up, expert_down, shared_gate, shared_up,
                           shared_down)
    res = run_bass_kernel_spmd(nc, in_maps, core_ids=list(range(N_CORES)))
    out = np.concatenate([np.asarray(res.results[c]["out"], np.float32)
                          for c in range(N_CORES)], axis=0)
    return out.reshape(np.asarray(hidden_states).shape)


# revision 44
# speedup vs baseline: 1.1591x; 1.0023x over previous
# DeepseekV3MoECalibrate Trainium2 kernel (8 NeuronCores, expert-parallel).
#
# Sharding: 32 experts -> 4 per core; shared expert split along the 2I=2048
# intermediate dim (256 per core, processed as one 2-i-tile pseudo-expert);
# tokens replicated; partial outputs summed with an on-device AllReduce.
#
# Numerics: router (sigmoid top-k selection) in fp32; expert/shared MLP
# matmuls in fp32r (full PE rate at N=512, ~1.6e-4 rel err); combine weights
# applied before the down-projection so each entry's contribution is a plain
# sum accumulated in SBUF.
#
# Schedule: weight transposition for entry k+1 is software-pipelined into
# entry k's down-projection (stage-3) matmul stream so the PE never crawls
# through an evacuation-gated transpose phase.
from contextlib import ExitStack

import numpy as np

import concourse.bass as bass
import concourse.tile as tile
from concourse import bacc, mybir
from concourse.masks import make_identity

F32 = mybir.dt.float32
F32R = mybir.dt.float32r
AF = mybir.ActivationFunctionType
OP = mybir.AluOpType
AX = mybir.AxisListType

N_CORES = 8
T, H, I, E = 1024, 1024, 512, 32
E_LOC = E // N_CORES          # 4 experts per core
ISH = 2 * I // N_CORES        # 256 shared-intermediate rows per core
TT = T // 128                 # 8 token tiles
HK = H // 128                 # 8 h k-tiles
IK = I // 128                 # 4 i-tiles per expert
SK = ISH // 128               # 2 shared i-tiles
NH = H // 512                 # 2 h halves (stage-3 rhs)
TH = T // 512                 # 2 t halves (stage-1 rhs)


def build_module(use_collective=True, num_devices=N_CORES):
    nc = bacc.Bacc("TRN2", target_bir_lowering=False, debug=False,
                   num_devices=num_devices)

    x_d = nc.dram_tensor("x", [T, H], F32, kind="ExternalInput")
    gw_d = nc.dram_tensor("gw", [E, H], F32, kind="ExternalInput")
    gb_d = nc.dram_tensor("gb", [1, E], F32, kind="ExternalInput")
    wsel_d = nc.dram_tensor("wsel", [E, E_LOC], F32, kind="ExternalInput")
    eg_d = nc.dram_tensor("eg", [E_LOC, I, H], F32, kind="ExternalInput")
    eu_d = nc.dram_tensor("eu", [E_LOC, I, H], F32, kind="ExternalInput")
    ed_d = nc.dram_tensor("ed", [E_LOC, H, I], F32, kind="ExternalInput")
    sg_d = nc.dram_tensor("sg", [ISH, H], F32, kind="ExternalInput")
    su_d = nc.dram_tensor("su", [ISH, H], F32, kind="ExternalInput")
    sd_d = nc.dram_tensor("sd", [H, ISH], F32, kind="ExternalInput")
    out_rows = T // num_devices if use_collective else T
    out_d = nc.dram_tensor("out", [out_rows, H], F32, kind="ExternalOutput")

    with tile.TileContext(nc) as tc, ExitStack() as ctx:
        const = ctx.enter_context(tc.tile_pool(name="const", bufs=1))
        sbr = ctx.enter_context(tc.tile_pool(name="router", bufs=2))
        xpool = ctx.enter_context(tc.tile_pool(name="xt", bufs=1))
        wg_pool = ctx.enter_context(tc.tile_pool(name="wgt", bufs=8))
        wu_pool = ctx.enter_context(tc.tile_pool(name="wut", bufs=8))
        wd_pool = ctx.enter_context(tc.tile_pool(name="wdt", bufs=4))
        nat_pool = ctx.enter_context(tc.tile_pool(name="nat", bufs=3))
        ats_pool = ctx.enter_context(tc.tile_pool(name="ats", bufs=4))
        tmp_pool = ctx.enter_context(tc.tile_pool(name="tmp", bufs=2))
        acc_pool = ctx.enter_context(tc.tile_pool(name="acc", bufs=1))
        dram = ctx.enter_context(tc.tile_pool(name="dram", bufs=1, space="DRAM"))

        ps_a = ctx.enter_context(tc.tile_pool(name="ps_a", bufs=2, space="PSUM"))
        ps_s = ctx.enter_context(tc.tile_pool(name="ps_s", bufs=4, space="PSUM"))
        ps_o = ctx.enter_context(tc.tile_pool(name="ps_o", bufs=2, space="PSUM"))

        ident_f = const.tile([128, 128], F32, name="ident_f")
        make_identity(nc, ident_f[:])
        ident_r = const.tile([128, 128], F32R, name="ident_r")
        nc.vector.tensor_copy(ident_r[:], ident_f[:])

        # ------------- router prep ------------------------------------------
        gw_sb = sbr.tile([E, H], F32, name="gw_sb")
        nc.gpsimd.dma_start(gw_sb[:], gw_d[:])
        gb_sb = sbr.tile([1, E], F32, name="gb_sb")
        nc.gpsimd.dma_start(gb_sb[:], gb_d[:])
        wsel_sb = sbr.tile([E, E_LOC], F32R, name="wsel_sb")
        nc.gpsimd.dma_start(wsel_sb[:], wsel_d[:].bitcast(F32R))

        gwT = [sbr.tile([128, E], F32, name=f"gwT{h}") for h in range(HK)]
        for ht in range(HK):
            p = ps_a.tile([128, 512], F32, name=f"gwp{ht}", tag="ps_a")
            nc.tensor.transpose(p[:, 0:E], gw_sb[:, ht * 128:(ht + 1) * 128],
                                ident_f[0:E, 0:E])
            nc.vector.tensor_copy(gwT[ht][:], p[:, 0:E])

        ones_f = const.tile([1, 128], F32, name="ones_f")
        nc.vector.memset(ones_f[:], 1.0)
        bias_bc = sbr.tile([128, E], F32, name="bias_bc")
        pb = ps_a.tile([128, 512], F32, name="pb", tag="ps_a")
        nc.tensor.matmul(pb[:, 0:E], ones_f[:], gb_sb[:], start=True, stop=True)
        nc.vector.tensor_copy(bias_bc[:], pb[:, 0:E])

        # ------------- persistent activations / accumulators -----------------
        xT_r = xpool.tile([128, HK * T], F32R, name="xTr")
        xview = xT_r[:].rearrange("p (h t) -> p h t", h=HK)
        wT_r = sbr.tile([E, T], F32R, name="wT_r")
        lgs = [sbr.tile([128, E], F32, name=f"lgs{tt}") for tt in range(TT)]
        out_acc = [acc_pool.tile([128, H], F32, name=f"oacc{tt}")
                   for tt in range(TT)]

        # ------------- weight prep (load + transpose), batched ---------------
        def make_prep(ei, kind, idx, ike):
            """Allocate destination tiles; return (wgTd, wdT, batches) where
            each batch closure emits a couple of PE transposes + one copy."""
            wgTd = {}
            batches = []
            for gu in ("g", "u"):
                pool = wg_pool if gu == "g" else wu_pool
                tagw = "wgt" if gu == "g" else "wut"
                dst = [pool.tile([128, ike * 128], F32R,
                                 name=f"w{gu}T{ei}_{h}", tag=tagw)
                       for h in range(HK)]
                wgTd[gu] = dst
                ihalf = 2 if kind == "expert" else 1
                per = ike // ihalf
                for ih in range(ihalf):
                    state = {}

                    def load_half(gu=gu, ih=ih, per=per, state=state):
                        natm = [nat_pool.tile([128, H], F32R,
                                              name=f"nat{ei}{gu}{ih}_{i}",
                                              tag="natgu", bufs=3)
                                for i in range(per)]
                        state["natm"] = natm
                        for i in range(per):
                            it = ih * per + i
                            if kind == "expert":
                                mat = eg_d if gu == "g" else eu_d
                                src = mat[idx, it * 128:(it + 1) * 128, :]
                            else:
                                mat = sg_d if gu == "g" else su_d
                                src = mat[it * 128:(it + 1) * 128, :]
                            nc.sync.dma_start(natm[i][:], src.bitcast(F32R))

                    for ht in range(HK):
                        def b(gu=gu, ih=ih, ht=ht, per=per, state=state,
                              dst=dst, lh=load_half):
                            if ht == 0:
                                lh()
                            natm = state["natm"]
                            p = ps_a.tile([128, 512], F32,
                                          name=f"wp{ei}{gu}{ih}{ht}", tag="ps_a")
                            for i in range(per):
                                nc.tensor.transpose(
                                    p[:, i * 128:(i + 1) * 128].bitcast(F32R),
                                    natm[i][:, ht * 128:(ht + 1) * 128],
                                    ident_r[:])
                            nc.scalar.copy(
                                dst[ht][:, ih * per * 128:(ih + 1) * per * 128],
                                p[:, 0:per * 128].bitcast(F32R))
                        batches.append(b)

            wdT = [wd_pool.tile([128, H], F32R, name=f"wdT{ei}_{i}", tag="wdt")
                   for i in range(ike)]
            for htq in range(HK // 4):
                state = {}

                def load_dn(htq=htq, state=state):
                    dn_nat = [nat_pool.tile([128, ike * 128], F32R,
                                            name=f"dnat{ei}_{htq}_{j}",
                                            tag="natdn", bufs=6)
                              for j in range(4)]
                    state["dn"] = dn_nat
                    for j in range(4):
                        ht = htq * 4 + j
                        if kind == "expert":
                            src = ed_d[idx, ht * 128:(ht + 1) * 128, :]
                        else:
                            src = sd_d[ht * 128:(ht + 1) * 128, :]
                        nc.sync.dma_start(dn_nat[j][:], src.bitcast(F32R))

                for it in range(ike):
                    def b(htq=htq, it=it, state=state, wdT=wdT, ld=load_dn):
                        if it == 0:
                            ld()
                        dn_nat = state["dn"]
                        p = ps_a.tile([128, 512], F32,
                                      name=f"dp{ei}_{htq}_{it}", tag="ps_a")
                        for j in range(4):
                            nc.tensor.transpose(
                                p[:, j * 128:(j + 1) * 128].bitcast(F32R),
                                dn_nat[j][:, it * 128:(it + 1) * 128],
                                ident_r[:])
                        nc.vector.tensor_copy(
                            wdT[it][:, htq * 512:(htq + 1) * 512].bitcast(F32R),
                            p[:].bitcast(F32R))
                    batches.append(b)
            return wgTd, wdT, batches

        # ------------- stage 1 / stage 3 -------------------------------------
        def stage1(ei, kind, ike, wgTd, wb):
            aTs = [ats_pool.tile([128, T], F32R, name=f"aTs{ei}_{i}", tag="ats")
                   for i in range(ike)]
            for th in range(TH):
                for it in range(ike):
                    gp = ps_s.tile([128, 512], F32, name=f"gp{ei}_{it}_{th}",
                                   tag="ps_s")
                    up = ps_s.tile([128, 512], F32, name=f"up{ei}_{it}_{th}",
                                   tag="ps_s")
                    for ht in range(HK):
                        nc.tensor.matmul(
                            gp[:], wgTd["g"][ht][:, it * 128:(it + 1) * 128],
                            xT_r[:, ht * T + th * 512: ht * T + (th + 1) * 512],
                            start=(ht == 0), stop=(ht == HK - 1))
                    for ht in range(HK):
                        nc.tensor.matmul(
                            up[:], wgTd["u"][ht][:, it * 128:(it + 1) * 128],
                            xT_r[:, ht * T + th * 512: ht * T + (th + 1) * 512],
                            start=(ht == 0), stop=(ht == HK - 1))
                    sg_t = tmp_pool.tile([128, 512], F32, name=f"sl{ei}{it}{th}",
                                         tag="silu", bufs=3)
                    nc.scalar.activation(sg_t[:], gp[:], AF.Silu)
                    dst = aTs[it][:, th * 512:(th + 1) * 512].bitcast(F32R)
                    if kind == "expert":
                        nc.vector.tensor_tensor(sg_t[:], sg_t[:], up[:], OP.mult)
                        nc.vector.tensor_tensor(
                            dst, sg_t[:], wb[:, th * 512:(th + 1) * 512],
                            OP.mult)
                    else:
                        nc.vector.tensor_tensor(dst, sg_t[:], up[:], OP.mult)
            return aTs

        def stage3_groups(ei, ike, aTs, wdT):
            groups = []
            for tt in range(TT):
                for hh in range(NH):
                    def g(tt=tt, hh=hh):
                        op = ps_o.tile([128, 512], F32, name=f"op{ei}_{tt}_{hh}",
                                       tag="ps_o")
                        for it in range(ike):
                            nc.tensor.matmul(
                                op[:], aTs[it][:, tt * 128:(tt + 1) * 128],
                                wdT[it][:, hh * 512:(hh + 1) * 512],
                                start=(it == 0), stop=(it == ike - 1))
                        dst = out_acc[tt][:, hh * 512:(hh + 1) * 512]
                        if ei == 0:
                            nc.vector.tensor_copy(dst, op[:])
                        else:
                            nc.vector.tensor_tensor(dst, dst, op[:], OP.add)
                    groups.append(g)
            return groups

        def emit_interleaved(groups, batches, front=2):
            bi = 0
            n = len(groups)
            for gi, g in enumerate(groups):
                g()
                want = min(len(batches), (gi + 1) * len(batches) * front // n)
                while bi < want:
                    batches[bi]()
                    bi += 1
            while bi < len(batches):
                batches[bi]()
                bi += 1

        # ------------- routing ------------------------------------------------
        def do_routing(tt):
            S = sbr.tile([128, E], F32, name=f"S{tt}", tag="S")
            nc.scalar.activation(S[:], lgs[tt][:], AF.Sigmoid)
            SC = sbr.tile([128, E], F32, name=f"SC{tt}", tag="SC")
            nc.vector.tensor_tensor(SC[:], S[:], bias_bc[:], OP.add)
            topg = sbr.tile([128, E], F32, name=f"topg{tt}", tag="topg")
            for g in range(4):
                nc.vector.max(topg[:, 8 * g:8 * g + 8], SC[:, 8 * g:8 * g + 8])
            gs8 = sbr.tile([128, 8], F32, name=f"gs8{tt}", tag="gs8")
            nc.vector.memset(gs8[:], -1e30)
            tg = topg[:].rearrange("p (g k) -> p g k", k=8)
            nc.vector.tensor_tensor(gs8[:, 0:4], tg[:, :, 0], tg[:, :, 1], OP.add)
            gtop = sbr.tile([128, 8], F32, name=f"gtop{tt}", tag="gtop")
            nc.vector.max(gtop[:], gs8[:])
            gmask = sbr.tile([128, 4], F32, name=f"gmask{tt}", tag="gmask")
            nc.vector.tensor_scalar(gmask[:], gs8[:, 0:4], gtop[:, 1:2], None,
                                    OP.is_ge)
            SCm = sbr.tile([128, E], F32, name=f"SCm{tt}", tag="SCm")
            nc.vector.tensor_tensor(
                SCm[:].rearrange("p (g k) -> p g k", k=8),
                SC[:].rearrange("p (g k) -> p g k", k=8),
                gmask[:].rearrange("p (g k) -> p g k", k=1).broadcast_to(
                    [128, 4, 8]),
                OP.mult)
            etop = sbr.tile([128, 8], F32, name=f"etop{tt}", tag="etop")
            nc.vector.max(etop[:], SCm[:])
            sel = sbr.tile([128, E], F32, name=f"sel{tt}", tag="sel")
            nc.vector.tensor_scalar(sel[:], SCm[:], etop[:, 7:8], None, OP.is_ge)
            wr = sbr.tile([128, E], F32, name=f"wr{tt}", tag="wr")
            nc.vector.tensor_tensor(wr[:], S[:], sel[:], OP.mult)
            den = sbr.tile([128, 1], F32, name=f"den{tt}", tag="den")
            nc.vector.reduce_sum(den[:], wr[:], axis=AX.X)
            nc.vector.tensor_scalar(den[:], den[:], 1.0 / 2.5, None, OP.mult)
            dinv = sbr.tile([128, 1], F32, name=f"dinv{tt}", tag="dinv")
            nc.vector.reciprocal(dinv[:], den[:])
            wt = sbr.tile([128, E], F32, name=f"wt{tt}", tag="wt")
            nc.vector.tensor_scalar(wt[:], wr[:], dinv[:], None, OP.mult)
            tp = ps_a.tile([128, 512], F32, name=f"tw{tt}", tag="ps_a")
            nc.tensor.transpose(tp[0:E, 0:128], wt[:], ident_f[:])
            nc.vector.tensor_copy(wT_r[:, tt * 128:(tt + 1) * 128].bitcast(F32R),
                                  tp[0:E, 0:128].bitcast(F32R))

        wb_tiles = {}
        wsel_bcs = {}

        def wb_th(e, th):
            if e not in wsel_bcs:
                wselbc = tmp_pool.tile([E, 128], F32R, name=f"wsb{e}",
                                       tag="wselbc")
                nc.vector.tensor_copy(
                    wselbc[:], wsel_sb[:, e:e + 1].broadcast_to([E, 128]))
                wsel_bcs[e] = wselbc
            if e not in wb_tiles:
                wb_tiles[e] = tmp_pool.tile([128, T], F32, name=f"wbx{e}",
                                            tag="wb")
            wb = wb_tiles[e]
            p = ps_a.tile([128, 512], F32, name=f"wbp{e}_{th}", tag="ps_a")
            nc.tensor.matmul(p[:], wsel_bcs[e][:],
                             wT_r[:, th * 512:(th + 1) * 512],
                             start=True, stop=True)
            nc.vector.tensor_copy(wb[:, th * 512:(th + 1) * 512], p[:])

        def make_wb(e):
            wb_th(e, 0)
            wb_th(e, 1)

        # ================= emission schedule =================================
        # Phase X: x transposes + router logits, with expert-0's weight prep
        # interleaved.  Routing is split by token half so expert-0's stage-1
        # th=0 can start as soon as tokens 0..511 are routed.  The shared
        # entry runs LAST so its (pool-serialized) weight prep overlaps the
        # final expert instead of the congested startup window.
        e0_prep = make_prep(0, "expert", 0, IK)
        bi = 0
        for tt in range(TT):
            xn = nat_pool.tile([128, H], F32, name=f"xn{tt}", tag="xn", bufs=2)
            (nc.sync if tt % 2 == 0 else nc.gpsimd).dma_start(
                xn[:], x_d[tt * 128:(tt + 1) * 128, :])
            xfb = tmp_pool.tile([128, H], F32, name=f"xfb{tt}", tag="xfb")
            for hq in range(HK // 4):
                p = ps_s.tile([128, 512], F32, name=f"xp{tt}_{hq}", tag="ps_s")
                for j in range(4):
                    ht = hq * 4 + j
                    nc.tensor.transpose(
                        p[:, j * 128:(j + 1) * 128],
                        xn[:, ht * 128:(ht + 1) * 128], ident_f[:])
                nc.scalar.copy(xfb[:, hq * 512:(hq + 1) * 512], p[:])
                nc.gpsimd.tensor_copy(
                    xview[:, hq * 4:(hq + 1) * 4,
                          tt * 128:(tt + 1) * 128].bitcast(F32R),
                    xfb[:, hq * 512:(hq + 1) * 512]
                    .rearrange("p (h t) -> p h t", h=4).bitcast(F32R))

            lg = ps_a.tile([128, 512], F32, name=f"lg{tt}", tag="ps_a")
            for ht in range(HK):
                nc.tensor.matmul(lg[:, 0:E], xfb[:, ht * 128:(ht + 1) * 128],
                                 gwT[ht][:],
                                 start=(ht == 0), stop=(ht == HK - 1))
            nc.scalar.copy(lgs[tt][:], lg[:, 0:E])

            want = (tt + 1) * len(e0_prep[2]) // TT
            while bi < want:
                e0_prep[2][bi]()
                bi += 1

        for tt in range(TT // 2):
            do_routing(tt)
        wb_th(0, 0)
        wb_th(1, 0)
        for tt in range(TT // 2, TT):
            do_routing(tt)
        wb_th(0, 1)
        wb_th(1, 1)

        # Entry pipeline: stage-3 of entry k interleaves entry k+1's prep.
        order = [("expert", e, IK) for e in range(E_LOC - 1)] + \
                [("shared", 0, SK), ("expert", E_LOC - 1, IK)]
        prev = e0_prep
        for k, (kind, idx, ike) in enumerate(order):
            wgTd, wdT, _ = prev
            wb = wb_tiles.get(idx) if kind == "expert" else None
            aTs = stage1(k, kind, ike, wgTd, wb)
            if kind == "expert" and idx + 2 < E_LOC:
                make_wb(idx + 2)
            if k + 1 < len(order):
                knd, nidx, nike = order[k + 1]
                nxt = make_prep(k + 1, knd, nidx, nike)
            else:
                nxt = None
            emit_interleaved(stage3_groups(k, ike, aTs, wdT),
                             nxt[2] if nxt else [],
                             front=2 if k < len(order) - 2 else 1)
            prev = nxt

        # ------------- ReduceScatter + output -------------------------------
        # Each core keeps its 128-token shard of the summed output; the host
        # concatenates the 8 shards.  RS moves ~30% less wire traffic than an
        # AllReduce of the full [T, H].
        if use_collective:
            bin_t = dram.tile([T, H], F32, name="rsin")
            bout_t = dram.tile([out_rows, H], F32, name="rsout")
            for tt in range(TT):
                nc.sync.dma_start(bin_t[tt * 128:(tt + 1) * 128, :],
                                  out_acc[tt][:])
            nc.gpsimd.collective_compute(
                "ReduceScatter", OP.add,
                replica_groups=[list(range(num_devices))],
                ins=[bin_t.opt()], outs=[bout_t.opt()])
            nc.sync.dma_start(out_d[:], bout_t[:])
        else:
            for tt in range(TT):
                nc.sync.dma_start(out_d[tt * 128:(tt + 1) * 128, :],
                                  out_acc[tt][:])
    nc.compile()
    return nc


_NC_CACHE = {}


def _get_module():
    key = "spmd"
    if key not in _NC_CACHE:
        _NC_CACHE[key] = build_module(use_collective=True, num_devices=N_CORES)
    return _NC_CACHE[key]


def make_in_maps(hidden_states, gate_w, gate_bias, expert_gate, expert_up,
                 expert_down, shared_gate, shared_up, shared_down):
    x = np.ascontiguousarray(
        np.asarray(hidden_states, np.float32).reshape(T, H))
    gw = np.ascontiguousarray(np.asarray(gate_w, np.float32))
    gb = np.ascontiguousarray(np.asarray(gate_bias, np.float32).reshape(1, E))
    in_maps = []
    for c in range(N_CORES):
        lo, hi = c * E_LOC, (c + 1) * E_LOC
        sel = np.zeros((E, E_LOC), np.float32)
        for j in range(E_LOC):
            sel[lo + j, j] = 1.0
        in_maps.append({
            "x": x, "gw": gw, "gb": gb, "wsel": sel,
            "eg": np.ascontiguousarray(np.asarray(expert_gate, np.float32)[lo:hi]),
            "eu": np.ascontiguousarray(np.asarray(expert_up, np.float32)[lo:hi]),
            "ed": np.ascontiguousarray(np.asarray(expert_down, np.float32)[lo:hi]),
            "sg": np.ascontiguousarray(
                np.asarray(shared_gate, np.float32)[c * ISH:(c + 1) * ISH]),
            "su": np.ascontiguousarray(
                np.asarray(shared_up, np.float32)[c * ISH:(c + 1) * ISH]),
            "sd": np.ascontiguousarray(
                np.asarray(shared_down, np.float32)[:, c * ISH:(c + 1) * ISH]),
        })
    return in_maps


def kernel(hidden_states, gate_w, gate_bias, expert_gate, expert_up,
           expert_down, shared_gate, shared_up, shared_down):
    import os
    # The axon NTFF trace hook is absent in this container; make sure the
    # PJRT execute path never tries to use it.
    os.environ.setdefault("BASS_NEVER_TRACE", "1")
    from concourse.bass_utils import run_bass_kernel_spmd
    nc = _get_module()
    in_maps = make_in_maps(hidden_states, gate_w, gate_bias, expert_gate,
                           expert_up, expert_down, shared_gate, shared_up,
                           shared_down)
    res = run_bass_kernel_spmd(nc, in_maps, core_ids=list(range(N_CORES)))
    out = np.concatenate([np.asarray(res.results[c]["out"], np.float32)
                          for c in range(N_CORES)], axis=0)
    return out.reshape(np.asarray(hidden_states).shape)


# revision 45
# speedup vs baseline: 1.1749x; 1.0136x over previous
# DeepseekV3MoECalibrate Trainium2 kernel (8 NeuronCores, expert-parallel).
#
# Sharding: 32 experts -> 4 per core; shared expert split along the 2I=2048
# intermediate dim (256 per core, processed as one 2-i-tile pseudo-expert);
# tokens replicated; partial outputs summed with an on-device AllReduce.
#
# Numerics: router (sigmoid top-k selection) in fp32; expert/shared MLP
# matmuls in fp32r (full PE rate at N=512, ~1.6e-4 rel err); combine weights
# applied before the down-projection so each entry's contribution is a plain
# sum accumulated in SBUF.
#
# Schedule: weight transposition for entry k+1 is software-pipelined into
# entry k's down-projection (stage-3) matmul stream so the PE never crawls
# through an evacuation-gated transpose phase.
from contextlib import ExitStack

import numpy as np

import concourse.bass as bass
import concourse.tile as tile
from concourse import bacc, mybir
from concourse.masks import make_identity

F32 = mybir.dt.float32
F32R = mybir.dt.float32r
AF = mybir.ActivationFunctionType
OP = mybir.AluOpType
AX = mybir.AxisListType

N_CORES = 8
T, H, I, E = 1024, 1024, 512, 32
E_LOC = E // N_CORES          # 4 experts per core
ISH = 2 * I // N_CORES        # 256 shared-intermediate rows per core
TT = T // 128                 # 8 token tiles
HK = H // 128                 # 8 h k-tiles
IK = I // 128                 # 4 i-tiles per expert
SK = ISH // 128               # 2 shared i-tiles
NH = H // 512                 # 2 h halves (stage-3 rhs)
TH = T // 512                 # 2 t halves (stage-1 rhs)


def build_module(use_collective=True, num_devices=N_CORES):
    nc = bacc.Bacc("TRN2", target_bir_lowering=False, debug=False,
                   num_devices=num_devices)

    x_d = nc.dram_tensor("x", [T, H], F32, kind="ExternalInput")
    gw_d = nc.dram_tensor("gw", [E, H], F32, kind="ExternalInput")
    gb_d = nc.dram_tensor("gb", [1, E], F32, kind="ExternalInput")
    wsel_d = nc.dram_tensor("wsel", [E, E_LOC], F32, kind="ExternalInput")
    eg_d = nc.dram_tensor("eg", [E_LOC, I, H], F32, kind="ExternalInput")
    eu_d = nc.dram_tensor("eu", [E_LOC, I, H], F32, kind="ExternalInput")
    ed_d = nc.dram_tensor("ed", [E_LOC, H, I], F32, kind="ExternalInput")
    sg_d = nc.dram_tensor("sg", [ISH, H], F32, kind="ExternalInput")
    su_d = nc.dram_tensor("su", [ISH, H], F32, kind="ExternalInput")
    sd_d = nc.dram_tensor("sd", [H, ISH], F32, kind="ExternalInput")
    out_rows = T // num_devices if use_collective else T
    out_d = nc.dram_tensor("out", [out_rows, H], F32, kind="ExternalOutput")

    with tile.TileContext(nc) as tc, ExitStack() as ctx:
        const = ctx.enter_context(tc.tile_pool(name="const", bufs=1))
        sbr = ctx.enter_context(tc.tile_pool(name="router", bufs=2))
        xpool = ctx.enter_context(tc.tile_pool(name="xt", bufs=1))
        wg_pool = ctx.enter_context(tc.tile_pool(name="wgt", bufs=8))
        wu_pool = ctx.enter_context(tc.tile_pool(name="wut", bufs=8))
        wd_pool = ctx.enter_context(tc.tile_pool(name="wdt", bufs=4))
        nat_pool = ctx.enter_context(tc.tile_pool(name="nat", bufs=3))
        ats_pool = ctx.enter_context(tc.tile_pool(name="ats", bufs=4))
        tmp_pool = ctx.enter_context(tc.tile_pool(name="tmp", bufs=2))
        acc_pool = ctx.enter_context(tc.tile_pool(name="acc", bufs=1))
        dram = ctx.enter_context(tc.tile_pool(name="dram", bufs=1, space="DRAM"))

        ps_a = ctx.enter_context(tc.tile_pool(name="ps_a", bufs=2, space="PSUM"))
        ps_s = ctx.enter_context(tc.tile_pool(name="ps_s", bufs=4, space="PSUM"))
        ps_o = ctx.enter_context(tc.tile_pool(name="ps_o", bufs=2, space="PSUM"))

        ident_f = const.tile([128, 128], F32, name="ident_f")
        make_identity(nc, ident_f[:])
        ident_r = const.tile([128, 128], F32R, name="ident_r")
        nc.vector.tensor_copy(ident_r[:], ident_f[:])

        # ------------- router prep ------------------------------------------
        gw_sb = nat_pool.tile([E, H], F32, name="gw_sb", tag="xn", bufs=2)
        nc.gpsimd.dma_start(gw_sb[:], gw_d[:])
        gb_sb = sbr.tile([1, E], F32, name="gb_sb")
        nc.gpsimd.dma_start(gb_sb[:], gb_d[:])
        wsel_sb = sbr.tile([E, E_LOC], F32R, name="wsel_sb")
        nc.gpsimd.dma_start(wsel_sb[:], wsel_d[:].bitcast(F32R))

        gwT = [sbr.tile([128, E], F32, name=f"gwT{h}") for h in range(HK)]
        for ht in range(HK):
            p = ps_a.tile([128, 512], F32, name=f"gwp{ht}", tag="ps_a")
            nc.tensor.transpose(p[:, 0:E], gw_sb[:, ht * 128:(ht + 1) * 128],
                                ident_f[0:E, 0:E])
            nc.vector.tensor_copy(gwT[ht][:], p[:, 0:E])

        ones_f = const.tile([1, 128], F32, name="ones_f")
        nc.vector.memset(ones_f[:], 1.0)
        bias_bc = sbr.tile([128, E], F32, name="bias_bc")
        pb = ps_a.tile([128, 512], F32, name="pb", tag="ps_a")
        nc.tensor.matmul(pb[:, 0:E], ones_f[:], gb_sb[:], start=True, stop=True)
        nc.vector.tensor_copy(bias_bc[:], pb[:, 0:E])

        # ------------- persistent activations / accumulators -----------------
        xT_r = xpool.tile([128, HK * T], F32R, name="xTr")
        xview = xT_r[:].rearrange("p (h t) -> p h t", h=HK)
        wT_r = sbr.tile([E, T], F32R, name="wT_r")
        lgs = [sbr.tile([128, E], F32, name=f"lgs{tt}") for tt in range(TT)]
        out_acc = [acc_pool.tile([128, H], F32, name=f"oacc{tt}")
                   for tt in range(TT)]

        # ------------- weight prep (load + transpose), batched ---------------
        def make_prep(ei, kind, idx, ike):
            """Allocate destination tiles; return (wgTd, wdT, batches) where
            each batch closure emits a couple of PE transposes + one copy."""
            wgTd = {}
            batches = []
            for gu in ("g", "u"):
                pool = wg_pool if gu == "g" else wu_pool
                tagw = "wgt" if gu == "g" else "wut"
                dst = [pool.tile([128, ike * 128], F32R,
                                 name=f"w{gu}T{ei}_{h}", tag=tagw)
                       for h in range(HK)]
                wgTd[gu] = dst
                ihalf = 2 if kind == "expert" else 1
                per = ike // ihalf
                for ih in range(ihalf):
                    state = {}

                    def load_half(gu=gu, ih=ih, per=per, state=state):
                        natm = [nat_pool.tile([128, H], F32R,
                                              name=f"nat{ei}{gu}{ih}_{i}",
                                              tag="natgu", bufs=4)
                                for i in range(per)]
                        state["natm"] = natm
                        for i in range(per):
                            it = ih * per + i
                            if kind == "expert":
                                mat = eg_d if gu == "g" else eu_d
                                src = mat[idx, it * 128:(it + 1) * 128, :]
                            else:
                                mat = sg_d if gu == "g" else su_d
                                src = mat[it * 128:(it + 1) * 128, :]
                            nc.sync.dma_start(natm[i][:], src.bitcast(F32R))

                    for ht in range(HK):
                        def b(gu=gu, ih=ih, ht=ht, per=per, state=state,
                              dst=dst, lh=load_half):
                            if ht == 0:
                                lh()
                            natm = state["natm"]
                            p = ps_a.tile([128, 512], F32,
                                          name=f"wp{ei}{gu}{ih}{ht}", tag="ps_a")
                            for i in range(per):
                                nc.tensor.transpose(
                                    p[:, i * 128:(i + 1) * 128].bitcast(F32R),
                                    natm[i][:, ht * 128:(ht + 1) * 128],
                                    ident_r[:])
                            nc.scalar.copy(
                                dst[ht][:, ih * per * 128:(ih + 1) * per * 128],
                                p[:, 0:per * 128].bitcast(F32R))
                        batches.append(b)

            wdT = [wd_pool.tile([128, H], F32R, name=f"wdT{ei}_{i}", tag="wdt")
                   for i in range(ike)]
            for htq in range(HK // 4):
                state = {}

                def load_dn(htq=htq, state=state):
                    dn_nat = [nat_pool.tile([128, ike * 128], F32R,
                                            name=f"dnat{ei}_{htq}_{j}",
                                            tag="natdn", bufs=6)
                              for j in range(4)]
                    state["dn"] = dn_nat
                    for j in range(4):
                        ht = htq * 4 + j
                        if kind == "expert":
                            src = ed_d[idx, ht * 128:(ht + 1) * 128, :]
                        else:
                            src = sd_d[ht * 128:(ht + 1) * 128, :]
                        nc.sync.dma_start(dn_nat[j][:], src.bitcast(F32R))

                for it in range(ike):
                    def b(htq=htq, it=it, state=state, wdT=wdT, ld=load_dn):
                        if it == 0:
                            ld()
                        dn_nat = state["dn"]
                        p = ps_a.tile([128, 512], F32,
                                      name=f"dp{ei}_{htq}_{it}", tag="ps_a")
                        for j in range(4):
                            nc.tensor.transpose(
                                p[:, j * 128:(j + 1) * 128].bitcast(F32R),
                                dn_nat[j][:, it * 128:(it + 1) * 128],
                                ident_r[:])
                        nc.vector.tensor_copy(
                            wdT[it][:, htq * 512:(htq + 1) * 512].bitcast(F32R),
                            p[:].bitcast(F32R))
                    batches.append(b)
            return wgTd, wdT, batches

        # ------------- stage 1 / stage 3 -------------------------------------
        def stage1(ei, kind, ike, wgTd, wb):
            aTs = [ats_pool.tile([128, T], F32R, name=f"aTs{ei}_{i}", tag="ats")
                   for i in range(ike)]
            for th in range(TH):
                for it in range(ike):
                    gp = ps_s.tile([128, 512], F32, name=f"gp{ei}_{it}_{th}",
                                   tag="ps_s")
                    up = ps_s.tile([128, 512], F32, name=f"up{ei}_{it}_{th}",
                                   tag="ps_s")
                    for ht in range(HK):
                        nc.tensor.matmul(
                            gp[:], wgTd["g"][ht][:, it * 128:(it + 1) * 128],
                            xT_r[:, ht * T + th * 512: ht * T + (th + 1) * 512],
                            start=(ht == 0), stop=(ht == HK - 1))
                    for ht in range(HK):
                        nc.tensor.matmul(
                            up[:], wgTd["u"][ht][:, it * 128:(it + 1) * 128],
                            xT_r[:, ht * T + th * 512: ht * T + (th + 1) * 512],
                            start=(ht == 0), stop=(ht == HK - 1))
                    sg_t = tmp_pool.tile([128, 512], F32, name=f"sl{ei}{it}{th}",
                                         tag="silu", bufs=3)
                    nc.scalar.activation(sg_t[:], gp[:], AF.Silu)
                    dst = aTs[it][:, th * 512:(th + 1) * 512].bitcast(F32R)
                    if kind == "expert":
                        nc.vector.tensor_tensor(sg_t[:], sg_t[:], up[:], OP.mult)
                        nc.vector.tensor_tensor(
                            dst, sg_t[:], wb[:, th * 512:(th + 1) * 512],
                            OP.mult)
                    else:
                        nc.vector.tensor_tensor(dst, sg_t[:], up[:], OP.mult)
            return aTs

        def stage3_groups(ei, ike, aTs, wdT):
            groups = []
            for tt in range(TT):
                for hh in range(NH):
                    def g(tt=tt, hh=hh):
                        op = ps_o.tile([128, 512], F32, name=f"op{ei}_{tt}_{hh}",
                                       tag="ps_o")
                        for it in range(ike):
                            nc.tensor.matmul(
                                op[:], aTs[it][:, tt * 128:(tt + 1) * 128],
                                wdT[it][:, hh * 512:(hh + 1) * 512],
                                start=(it == 0), stop=(it == ike - 1))
                        dst = out_acc[tt][:, hh * 512:(hh + 1) * 512]
                        if ei == 0:
                            nc.vector.tensor_copy(dst, op[:])
                        else:
                            nc.vector.tensor_tensor(dst, dst, op[:], OP.add)
                    groups.append(g)
            return groups

        def emit_interleaved(groups, batches, front=2):
            bi = 0
            n = len(groups)
            for gi, g in enumerate(groups):
                g()
                want = min(len(batches), (gi + 1) * len(batches) * front // n)
                while bi < want:
                    batches[bi]()
                    bi += 1
            while bi < len(batches):
                batches[bi]()
                bi += 1

        # ------------- routing ------------------------------------------------
        def do_routing(tt):
            S = sbr.tile([128, E], F32, name=f"S{tt}", tag="S")
            nc.scalar.activation(S[:], lgs[tt][:], AF.Sigmoid)
            SC = sbr.tile([128, E], F32, name=f"SC{tt}", tag="SC")
            nc.vector.tensor_tensor(SC[:], S[:], bias_bc[:], OP.add)
            topg = sbr.tile([128, E], F32, name=f"topg{tt}", tag="topg")
            for g in range(4):
                nc.vector.max(topg[:, 8 * g:8 * g + 8], SC[:, 8 * g:8 * g + 8])
            gs8 = sbr.tile([128, 8], F32, name=f"gs8{tt}", tag="gs8")
            nc.vector.memset(gs8[:], -1e30)
            tg = topg[:].rearrange("p (g k) -> p g k", k=8)
            nc.vector.tensor_tensor(gs8[:, 0:4], tg[:, :, 0], tg[:, :, 1], OP.add)
            gtop = sbr.tile([128, 8], F32, name=f"gtop{tt}", tag="gtop")
            nc.vector.max(gtop[:], gs8[:])
            gmask = sbr.tile([128, 4], F32, name=f"gmask{tt}", tag="gmask")
            nc.vector.tensor_scalar(gmask[:], gs8[:, 0:4], gtop[:, 1:2], None,
                                    OP.is_ge)
            SCm = sbr.tile([128, E], F32, name=f"SCm{tt}", tag="SCm")
            nc.vector.tensor_tensor(
                SCm[:].rearrange("p (g k) -> p g k", k=8),
                SC[:].rearrange("p (g k) -> p g k", k=8),
                gmask[:].rearrange("p (g k) -> p g k", k=1).broadcast_to(
                    [128, 4, 8]),
                OP.mult)
            etop = sbr.tile([128, 8], F32, name=f"etop{tt}", tag="etop")
            nc.vector.max(etop[:], SCm[:])
            sel = sbr.tile([128, E], F32, name=f"sel{tt}", tag="sel")
            nc.vector.tensor_scalar(sel[:], SCm[:], etop[:, 7:8], None, OP.is_ge)
            wr = sbr.tile([128, E], F32, name=f"wr{tt}", tag="wr")
            nc.vector.tensor_tensor(wr[:], S[:], sel[:], OP.mult)
            den = sbr.tile([128, 1], F32, name=f"den{tt}", tag="den")
            nc.vector.reduce_sum(den[:], wr[:], axis=AX.X)
            nc.vector.tensor_scalar(den[:], den[:], 1.0 / 2.5, None, OP.mult)
            dinv = sbr.tile([128, 1], F32, name=f"dinv{tt}", tag="dinv")
            nc.vector.reciprocal(dinv[:], den[:])
            wt = sbr.tile([128, E], F32, name=f"wt{tt}", tag="wt")
            nc.vector.tensor_scalar(wt[:], wr[:], dinv[:], None, OP.mult)
            tp = ps_a.tile([128, 512], F32, name=f"tw{tt}", tag="ps_a")
            nc.tensor.transpose(tp[0:E, 0:128], wt[:], ident_f[:])
            nc.vector.tensor_copy(wT_r[:, tt * 128:(tt + 1) * 128].bitcast(F32R),
                                  tp[0:E, 0:128].bitcast(F32R))

        wb_tiles = {}
        wsel_bcs = {}

        def wb_th(e, th):
            if e not in wsel_bcs:
                wselbc = tmp_pool.tile([E, 128], F32R, name=f"wsb{e}",
                                       tag="wselbc")
                nc.vector.tensor_copy(
                    wselbc[:], wsel_sb[:, e:e + 1].broadcast_to([E, 128]))
                wsel_bcs[e] = wselbc
            if e not in wb_tiles:
                wb_tiles[e] = tmp_pool.tile([128, T], F32, name=f"wbx{e}",
                                            tag="wb")
            wb = wb_tiles[e]
            p = ps_a.tile([128, 512], F32, name=f"wbp{e}_{th}", tag="ps_a")
            nc.tensor.matmul(p[:], wsel_bcs[e][:],
                             wT_r[:, th * 512:(th + 1) * 512],
                             start=True, stop=True)
            nc.vector.tensor_copy(wb[:, th * 512:(th + 1) * 512], p[:])

        def make_wb(e):
            wb_th(e, 0)
            wb_th(e, 1)

        # ================= emission schedule =================================
        # Phase X: x transposes + router logits, with expert-0's weight prep
        # interleaved.  Routing is split by token half so expert-0's stage-1
        # th=0 can start as soon as tokens 0..511 are routed.  The shared
        # entry runs LAST so its (pool-serialized) weight prep overlaps the
        # final expert instead of the congested startup window.
        e0_prep = make_prep(0, "expert", 0, IK)
        bi = 0
        for tt in range(TT):
            xn = nat_pool.tile([128, H], F32, name=f"xn{tt}", tag="xn", bufs=2)
            (nc.sync if tt % 2 == 0 else nc.gpsimd).dma_start(
                xn[:], x_d[tt * 128:(tt + 1) * 128, :])
            xfb = tmp_pool.tile([128, H], F32, name=f"xfb{tt}", tag="xfb")
            for hq in range(HK // 4):
                p = ps_s.tile([128, 512], F32, name=f"xp{tt}_{hq}", tag="ps_s")
                for j in range(4):
                    ht = hq * 4 + j
                    nc.tensor.transpose(
                        p[:, j * 128:(j + 1) * 128],
                        xn[:, ht * 128:(ht + 1) * 128], ident_f[:])
                nc.scalar.copy(xfb[:, hq * 512:(hq + 1) * 512], p[:])
                nc.gpsimd.tensor_copy(
                    xview[:, hq * 4:(hq + 1) * 4,
                          tt * 128:(tt + 1) * 128].bitcast(F32R),
                    xfb[:, hq * 512:(hq + 1) * 512]
                    .rearrange("p (h t) -> p h t", h=4).bitcast(F32R))

            lg = ps_a.tile([128, 512], F32, name=f"lg{tt}", tag="ps_a")
            for ht in range(HK):
                nc.tensor.matmul(lg[:, 0:E], xfb[:, ht * 128:(ht + 1) * 128],
                                 gwT[ht][:],
                                 start=(ht == 0), stop=(ht == HK - 1))
            nc.scalar.copy(lgs[tt][:], lg[:, 0:E])

            want = (tt + 1) * len(e0_prep[2]) // TT
            while bi < want:
                e0_prep[2][bi]()
                bi += 1

        for tt in range(TT // 2):
            do_routing(tt)
        wb_th(0, 0)
        wb_th(1, 0)
        for tt in range(TT // 2, TT):
            do_routing(tt)
        wb_th(0, 1)
        wb_th(1, 1)

        # Entry pipeline: stage-3 of entry k interleaves entry k+1's prep.
        order = [("expert", e, IK) for e in range(E_LOC - 1)] + \
                [("shared", 0, SK), ("expert", E_LOC - 1, IK)]
        prev = e0_prep
        for k, (kind, idx, ike) in enumerate(order):
            wgTd, wdT, _ = prev
            wb = wb_tiles.get(idx) if kind == "expert" else None
            aTs = stage1(k, kind, ike, wgTd, wb)
            if kind == "expert" and idx + 2 < E_LOC:
                make_wb(idx + 2)
            if k + 1 < len(order):
                knd, nidx, nike = order[k + 1]
                nxt = make_prep(k + 1, knd, nidx, nike)
            else:
                nxt = None
            emit_interleaved(stage3_groups(k, ike, aTs, wdT),
                             nxt[2] if nxt else [],
                             front=2 if k < len(order) - 2 else 1)
            prev = nxt

        # ------------- ReduceScatter + output -------------------------------
        # Each core keeps its 128-token shard of the summed output; the host
        # concatenates the 8 shards.  RS moves ~30% less wire traffic than an
        # AllReduce of the full [T, H].
        if use_collective:
            bin_t = dram.tile([T, H], F32, name="rsin")
            bout_t = dram.tile([out_rows, H], F32, name="rsout")
            for tt in range(TT):
                nc.sync.dma_start(bin_t[tt * 128:(tt + 1) * 128, :],
                                  out_acc[tt][:])
            nc.gpsimd.collective_compute(
                "ReduceScatter", OP.add,
                replica_groups=[list(range(num_devices))],
                ins=[bin_t.opt()], outs=[bout_t.opt()])
            nc.sync.dma_start(out_d[:], bout_t[:])
        else:
            for tt in range(TT):
                nc.sync.dma_start(out_d[tt * 128:(tt + 1) * 128, :],
                                  out_acc[tt][:])
    nc.compile()
    return nc


_NC_CACHE = {}


def _get_module():
    key = "spmd"
    if key not in _NC_CACHE:
        _NC_CACHE[key] = build_module(use_collective=True, num_devices=N_CORES)
    return _NC_CACHE[key]


def make_in_maps(hidden_states, gate_w, gate_bias, expert_gate, expert_up,
                 expert_down, shared_gate, shared_up, shared_down):
    x = np.ascontiguousarray(
        np.asarray(hidden_states, np.float32).reshape(T, H))
    gw = np.ascontiguousarray(np.asarray(gate_w, np.float32))
    gb = np.ascontiguousarray(np.asarray(gate_bias, np.float32).reshape(1, E))
    in_maps = []
    for c in range(N_CORES):
        lo, hi = c * E_LOC, (c + 1) * E_LOC
        sel = np.zeros((E, E_LOC), np.float32)
        for j in range(E_LOC):
            sel[lo + j, j] = 1.0
        in_maps.append({
            "x": x, "gw": gw, "gb": gb, "wsel": sel,
            "eg": np.ascontiguousarray(np.asarray(expert_gate, np.float32)[lo:hi]),
            "eu": np.ascontiguousarray(np.asarray(expert_up, np.float32)[lo:hi]),
            "ed": np.ascontiguousarray(np.asarray(expert_down, np.float32)[lo:hi]),
            "sg": np.ascontiguousarray(
                np.asarray(shared_gate, np.float32)[c * ISH:(c + 1) * ISH]),
            "su": np.ascontiguousarray(
                np.asarray(shared_up, np.float32)[c * ISH:(c + 1) * ISH]),
            "sd": np.ascontiguousarray(
                np.asarray(shared_down, np.float32)[:, c * ISH:(c + 1) * ISH]),
        })
    return in_maps


def kernel(hidden_states, gate_w, gate_bias, expert_gate, expert_up,
           expert_down, shared_gate, shared_up, shared_down):
    import os
    # The axon NTFF trace hook is absent in this container; make sure the
    # PJRT execute path never tries to use it.
    os.environ.setdefault("BASS_NEVER_TRACE", "1")
    from concourse.bass_utils import run_bass_kernel_spmd
    nc = _get_module()
    in_maps = make_in_maps(hidden_states, gate_w, gate_bias, expert_gate,
                           expert_up, expert_down, shared_gate, shared_up,
                           shared_down)
    res = run_bass_kernel_spmd(nc, in_maps, core_ids=list(range(N_CORES)))
    out = np.concatenate([np.asarray(res.results[c]["out"], np.float32)
                          for c in range(N_CORES)], axis=0)
    return out.reshape(np.asarray(hidden_states).shape)
